# revision 8
# baseline (speedup 1.0000x reference)
"""DemonsOrientation loss kernel for Trainium2 (8 NeuronCores).

Math (reference): six separable 3x3x3 Sobel-style gradients of M and S,
demons orientation angles arctan(Ux/Uz), arctan(Uy/Uz), flow orientation
angles, and the mean of squared angle differences.

Decomposition per gradient (cross-correlation, padding=1):
  kx = box_d  (x) smooth_h (x) diff_w
  ky = box_d  (x) diff_h   (x) smooth_w
  kz = diff_d (x) smooth_h (x) box_w
with box = [1,1,1], smooth = [1,2,1], diff = [-1,0,1].

Sharding: D=160 split 8 ways (20 slices/core + 1-slice halo, sliced from
the full input on the host; no device-side exchange needed). Within a
core, H=192 splits into chunk A (input h 0..127 -> output h 0..126) and
chunk B (input h 126..191 -> output h 127..191).

Per core:
  TensorE (bf16): banded matmuls apply the h-stencil; PSUM accumulation
    over three d-shifted rhs views applies the d-stencil. Produces
    P1 = smooth_h(box_d X), P2 = diff_h(box_d X), P3 = smooth_h(diff_d X).
  ScalarE: evacuates P* to SBUF as bf16; squares; arctan; error
    squares with accum_out (per-partition partial sums).
  DVE/GPSIMD: w-stencils (shifted-AP adds over two stencil rounds at
    once) and the bf16 pointwise chain. With q = denom_S/denom_M:
    t_xz = (Sx + q Mx)/(Sz + q Mz + eps), algebraically equal to the
    reference Ux/(Uz+1e-10) up to the vanishing stabilizer. The
    approx-reciprocal custom DVE op needs fp32, so denom_M, Nz and
    fz+eps are materialized fp32.
  Host: sums the per-partition accumulators (fp64) / voxel count.

Everything is bf16 except the PSUM accumulation, the reciprocal islands
and the error accumulators; measured end-to-end rel-err vs the fp32
reference is ~2e-4 (the per-voxel rounding noise averages out over the
4.9M-voxel mean).
"""

import numpy as np
import ml_dtypes
from contextlib import ExitStack

import concourse.bass as bass
import concourse.bacc as bacc
import concourse.tile as tile
from concourse import mybir
from concourse.bass_utils import run_bass_kernel_spmd

F32 = mybir.dt.float32
BF16 = mybir.dt.bfloat16
NPBF = ml_dtypes.bfloat16

D, H, W = 160, 192, 160
NCORES = 8
DL = D // NCORES          # 20 out slices per core
DS = DL + 2               # slab d extent (with halo)
WH = W + 2                # w extent with halo
G = 2                     # d-slices per stencil round (PSUM bank limit)
G2 = 4                    # d-slices per pointwise round
NR = DL // G              # stencil rounds
NPW = DL // G2            # pointwise rounds
RPP = G2 // G             # stencil rounds per pointwise round

AIN, AOUT = 128, 127      # chunk A: input h 0..127 -> out h 0..126
BIN, BOUT = 66, 65        # chunk B: input h 126..191 -> out h 127..191

EPS = 1e-10
NACC = 2 * NPW


def _band_matrices():
    """lhsT band matrices for the h-stencil matmuls (out = lhsT.T @ rhs)."""
    BsA = np.zeros((AIN, AOUT), np.float32)
    BdA = np.zeros((AIN, AOUT), np.float32)
    for m in range(AOUT):
        for dk, c in ((-1, 1.0), (0, 2.0), (1, 1.0)):
            if 0 <= m + dk < AIN:
                BsA[m + dk, m] = c
        for dk, c in ((-1, -1.0), (1, 1.0)):
            if 0 <= m + dk < AIN:
                BdA[m + dk, m] = c
    BsB = np.zeros((BIN, BOUT), np.float32)
    BdB = np.zeros((BIN, BOUT), np.float32)
    for m in range(BOUT):
        for dk, c in ((-1, 1.0), (0, 2.0), (1, 1.0)):
            if 0 <= m + 1 + dk < BIN:
                BsB[m + 1 + dk, m] = c
        for dk, c in ((-1, -1.0), (1, 1.0)):
            if 0 <= m + 1 + dk < BIN:
                BdB[m + 1 + dk, m] = c
    return BsA, BdA, -BsA, BsB, BdB, -BsB


def _build_nc():
    nc = bacc.Bacc("TRN2")
    din = {}
    for nm, shp in (("gMA", [AIN, DS, WH]), ("gMB", [BIN, DS, WH]),
                    ("gSA", [AIN, DS, WH]), ("gSB", [BIN, DS, WH]),
                    ("gMBi", [BOUT, DL, W]), ("gSBi", [BOUT, DL, W]),
                    ("gFA", [AOUT, 3, DL, W]), ("gFB", [BOUT, 3, DL, W]),
                    ("gBsA", [AIN, AOUT]), ("gBdA", [AIN, AOUT]),
                    ("gBnA", [AIN, AOUT]), ("gBsB", [BIN, BOUT]),
                    ("gBdB", [BIN, BOUT]), ("gBnB", [BIN, BOUT])):
        din[nm] = nc.dram_tensor(nm, shp, BF16, kind="ExternalInput")
    accA = nc.dram_tensor("gaccA", [AOUT, NACC], F32, kind="ExternalOutput")
    accB = nc.dram_tensor("gaccB", [BOUT, NACC], F32, kind="ExternalOutput")

    AL = mybir.AluOpType
    AF = mybir.ActivationFunctionType

    with ExitStack() as ctx:
        tc = ctx.enter_context(tile.TileContext(nc))
        persist = ctx.enter_context(tc.tile_pool(name="persist", bufs=1))
        psum = ctx.enter_context(tc.tile_pool(name="psum", bufs=1, space="PSUM"))
        gpool = ctx.enter_context(tc.tile_pool(name="gpool", bufs=2))
        fpool = ctx.enter_context(tc.tile_pool(name="fpool", bufs=3))
        cpool = ctx.enter_context(tc.tile_pool(name="cpool", bufs=2))
        wpool = ctx.enter_context(tc.tile_pool(name="wpool", bufs=3))
        vpool = ctx.enter_context(tc.tile_pool(name="vpool", bufs=18))

        slab = {}
        for nm, pin in (("gMA", AIN), ("gMB", BIN), ("gSA", AIN), ("gSB", BIN)):
            st = persist.tile([pin, DS, WH], BF16, tag=nm, name=nm)
            nc.sync.dma_start(out=st, in_=din[nm][:, :, :])
            slab[nm] = st
        for nm in ("gMBi", "gSBi"):
            st = persist.tile([BOUT, DL, W], BF16, tag=nm, name=nm)
            nc.sync.dma_start(out=st, in_=din[nm][:, :, :])
            slab[nm] = st
        band = {}
        for nm, pin, pout in (("gBsA", AIN, AOUT), ("gBdA", AIN, AOUT),
                              ("gBnA", AIN, AOUT), ("gBsB", BIN, BOUT),
                              ("gBdB", BIN, BOUT), ("gBnB", BIN, BOUT)):
            bt = persist.tile([pin, pout], BF16, tag=nm, name=nm)
            nc.sync.dma_start(out=bt, in_=din[nm][:, :])
            band[nm] = bt
        acc_t = {
            "A": persist.tile([AOUT, NACC], F32, tag="accAt", name="accAt"),
            "B": persist.tile([BOUT, NACC], F32, tag="accBt", name="accBt"),
        }

        for pw in range(NPW):
            for ch in ("A", "B"):
                pout = AOUT if ch == "A" else BOUT
                Bs, Bd, Bn = band["gBs" + ch], band["gBd" + ch], band["gBn" + ch]
                msl, ssl = slab["gM" + ch], slab["gS" + ch]

                # c tiles: evacuated P1..P3, both stencil rounds, per X
                ct = {}
                for Xn in ("S", "M"):
                    for j in (1, 2, 3):
                        k = f"c{j}{Xn}"
                        ct[k] = cpool.tile([pout, G2, WH], BF16, tag=k, name=k)

                for rr in range(RPP):
                    s0 = (pw * RPP + rr) * G
                    dsl = slice(rr * G, rr * G + G)
                    for Xn, xsl in (("S", ssl), ("M", msl)):
                        p1 = psum.tile([pout, G, WH], F32, tag="p1" + Xn, name="p1" + Xn)
                        p2 = psum.tile([pout, G, WH], F32, tag="p2" + Xn, name="p2" + Xn)
                        p3 = psum.tile([pout, G, WH], F32, tag="p3" + Xn, name="p3" + Xn)
                        for i in (0, 1, 2):
                            rhs = xsl[:, s0 + i: s0 + i + G, :]
                            nc.tensor.matmul(p1, Bs, rhs, start=(i == 0), stop=(i == 2))
                        for i in (0, 1, 2):
                            rhs = xsl[:, s0 + i: s0 + i + G, :]
                            nc.tensor.matmul(p2, Bd, rhs, start=(i == 0), stop=(i == 2))
                        nc.tensor.matmul(p3, Bs, xsl[:, s0 + 2: s0 + 2 + G, :],
                                         start=True, stop=False)
                        nc.tensor.matmul(p3, Bn, xsl[:, s0: s0 + G, :],
                                         start=False, stop=True)
                        # evacuate PSUM -> SBUF bf16 (ScalarE, downcast)
                        nc.scalar.copy(ct["c1" + Xn][:, dsl, :], p1)
                        nc.scalar.copy(ct["c2" + Xn][:, dsl, :], p2)
                        nc.scalar.copy(ct["c3" + Xn][:, dsl, :], p3)

                # w stencils over the whole G2 block (N = G2*W)
                gt = {}
                for Xn in ("S", "M"):
                    c1, c2, c3 = ct["c1" + Xn], ct["c2" + Xn], ct["c3" + Xn]
                    for c in ("x", "y", "z"):
                        k = c + Xn
                        gt[k] = gpool.tile([pout, G2, W], BF16, tag="g" + k,
                                           name="g" + k)
                    # Gx = P1[w+1] - P1[w-1]   (4B-aligned shifts: 2x mode)
                    nc.vector.tensor_sub(gt["x" + Xn], c1[:, :, 2:WH], c1[:, :, 0:W])
                    # Gy = (2*P2[w] + P2[w-1]) + P2[w+1]
                    t1 = wpool.tile([pout, G2, W], BF16, tag="wt1", name="wt1")
                    nc.vector.scalar_tensor_tensor(
                        t1, c2[:, :, 1:W + 1], 2.0, c2[:, :, 0:W],
                        op0=AL.mult, op1=AL.add)
                    nc.vector.tensor_add(gt["y" + Xn], t1, c2[:, :, 2:WH])
                    # Gz = (P3[w-1] + P3[w]) + P3[w+1]
                    t2 = wpool.tile([pout, G2, W], BF16, tag="wt2", name="wt2")
                    nc.gpsimd.tensor_add(t2, c3[:, :, 0:W], c3[:, :, 1:W + 1])
                    nc.vector.tensor_add(gt["z" + Xn], t2, c3[:, :, 2:WH])

                # ---- pointwise on [pout, G2, W] ----
                def vt(tag, dt=BF16):
                    return vpool.tile([pout, G2, W], dt, tag="v", name=tag)

                d0 = pw * G2
                if ch == "A":
                    mi = msl[0:AOUT, d0 + 1: d0 + 1 + G2, 1:W + 1]
                    si = ssl[0:AOUT, d0 + 1: d0 + 1 + G2, 1:W + 1]
                else:
                    mi = slab["gMBi"][:, d0: d0 + G2, :]
                    si = slab["gSBi"][:, d0: d0 + G2, :]

                idf = vt("idf")
                nc.vector.tensor_sub(idf, mi, si)
                i2 = vt("i2")
                nc.scalar.activation(i2, idf, AF.Square)

                # denominators: squares split ACT/DVE, sums bf16
                sqxS, sqyS, sqzS = vt("sqxS"), vt("sqyS"), vt("sqzS")
                nc.scalar.activation(sqxS, gt["xS"], AF.Square)
                nc.vector.tensor_mul(sqyS, gt["yS"], gt["yS"])
                nc.scalar.activation(sqzS, gt["zS"], AF.Square)
                dS0 = vt("dS0")
                nc.gpsimd.tensor_add(dS0, sqxS, sqyS)
                dS1 = vt("dS1")
                nc.vector.tensor_add(dS1, dS0, sqzS)
                dS = vt("dS")
                nc.vector.scalar_tensor_tensor(dS, i2, EPS, dS1,
                                               op0=AL.add, op1=AL.add)
                sqxM, sqyM, sqzM = vt("sqxM"), vt("sqyM"), vt("sqzM")
                nc.scalar.activation(sqxM, gt["xM"], AF.Square)
                nc.vector.tensor_mul(sqyM, gt["yM"], gt["yM"])
                nc.scalar.activation(sqzM, gt["zM"], AF.Square)
                dM0 = vt("dM0")
                nc.gpsimd.tensor_add(dM0, sqxM, sqyM)
                dM1 = vt("dM1")
                nc.vector.tensor_add(dM1, dM0, sqzM)
                dM = vt("dM", F32)
                nc.vector.scalar_tensor_tensor(dM, i2, EPS, dM1,
                                               op0=AL.add, op1=AL.add)

                rdM = vt("rdM", F32)
                nc.vector.reciprocal_approx_fast(rdM, dM)
                q = vt("q")
                nc.gpsimd.tensor_mul(q, dS, rdM)

                mx, my, mz = vt("mx"), vt("my"), vt("mz")
                nc.vector.tensor_mul(mx, gt["xM"], q)
                nc.gpsimd.tensor_mul(my, gt["yM"], q)
                nc.vector.tensor_mul(mz, gt["zM"], q)
                nx, ny = vt("nx"), vt("ny")
                nc.vector.tensor_add(nx, mx, gt["xS"])
                nc.gpsimd.tensor_add(ny, my, gt["yS"])
                # sum first, add eps to the (possibly exactly-zero) sum:
                # with bf16 operands mz == -gzS happens at ~0.1% rate and
                # reciprocal_approx_fast(0) is NaN.
                nz0 = vt("nz0", F32)
                nc.vector.tensor_add(nz0, mz, gt["zS"])
                nz = vt("nz", F32)
                nc.vector.tensor_scalar_add(nz, nz0, 1e-12)
                rnz = vt("rnz", F32)
                nc.vector.reciprocal_approx_fast(rnz, nz)
                t1v = vt("t1v")
                nc.vector.tensor_mul(t1v, nx, rnz)
                t2v = vt("t2v")
                nc.gpsimd.tensor_mul(t2v, ny, rnz)
                a1, a2 = vt("a1"), vt("a2")
                nc.scalar.activation(a1, t1v, AF.Arctan)
                nc.scalar.activation(a2, t2v, AF.Arctan)

                # flow side
                ft = fpool.tile([pout, 3, G2, W], BF16, tag="flow", name="flow")
                fdr = din["gFA"] if ch == "A" else din["gFB"]
                nc.sync.dma_start(out=ft, in_=fdr[:, :, d0: d0 + G2, :])
                fze = vt("fze", F32)
                nc.vector.tensor_scalar_add(fze, ft[:, 2], EPS)
                rfz = vt("rfz", F32)
                nc.vector.reciprocal_approx_fast(rfz, fze)
                t3v = vt("t3v")
                nc.gpsimd.tensor_mul(t3v, ft[:, 0], rfz)
                t4v = vt("t4v")
                nc.gpsimd.tensor_mul(t4v, ft[:, 1], rfz)
                b1, b2 = vt("b1"), vt("b2")
                nc.scalar.activation(b1, t3v, AF.Arctan)
                nc.scalar.activation(b2, t4v, AF.Arctan)

                # error accumulation
                d1 = vt("d1")
                nc.vector.tensor_sub(d1, b1, a1)
                scr1 = vt("scr1")
                nc.scalar.activation(scr1, d1, AF.Square,
                                     accum_out=acc_t[ch][:, 2 * pw: 2 * pw + 1])
                d2 = vt("d2")
                nc.gpsimd.tensor_sub(d2, b2, a2)
                scr2 = vt("scr2")
                nc.scalar.activation(scr2, d2, AF.Square,
                                     accum_out=acc_t[ch][:, 2 * pw + 1: 2 * pw + 2])

        nc.sync.dma_start(out=accA[:, :], in_=acc_t["A"])
        nc.sync.dma_start(out=accB[:, :], in_=acc_t["B"])

    nc.compile()
    return nc


_NC_CACHE = None


def _get_nc():
    global _NC_CACHE
    if _NC_CACHE is None:
        _NC_CACHE = _build_nc()
    return _NC_CACHE


def _prep_inputs(M, S, flow):
    M3 = np.asarray(M, np.float32).reshape(D, H, W)
    S3 = np.asarray(S, np.float32).reshape(D, H, W)
    F3 = np.asarray(flow, np.float32).reshape(3, D, H, W)
    MP = np.zeros((D + 2, H, WH), NPBF)
    SP = np.zeros((D + 2, H, WH), NPBF)
    MP[1:D + 1, :, 1:W + 1] = M3.astype(NPBF)
    SP[1:D + 1, :, 1:W + 1] = S3.astype(NPBF)
    FBF = F3.astype(NPBF)
    BsA, BdA, BnA, BsB, BdB, BnB = _band_matrices()
    bands = {"gBsA": BsA.astype(NPBF), "gBdA": BdA.astype(NPBF),
             "gBnA": BnA.astype(NPBF), "gBsB": BsB.astype(NPBF),
             "gBdB": BdB.astype(NPBF), "gBnB": BnB.astype(NPBF)}
    in_maps = []
    for c in range(NCORES):
        msl = np.ascontiguousarray(MP[c * DL: c * DL + DS].transpose(1, 0, 2))
        ssl = np.ascontiguousarray(SP[c * DL: c * DL + DS].transpose(1, 0, 2))
        fsl = np.ascontiguousarray(
            FBF[:, c * DL: (c + 1) * DL].transpose(2, 0, 1, 3))
        in_maps.append({
            "gMA": msl[0:AIN], "gMB": np.ascontiguousarray(msl[H - BIN: H]),
            "gSA": ssl[0:AIN], "gSB": np.ascontiguousarray(ssl[H - BIN: H]),
            "gFA": fsl[0:AOUT], "gFB": np.ascontiguousarray(fsl[H - BOUT: H]),
            "gMBi": np.ascontiguousarray(msl[H - BOUT: H, 1:DS - 1, 1:W + 1]),
            "gSBi": np.ascontiguousarray(ssl[H - BOUT: H, 1:DS - 1, 1:W + 1]),
            **bands,
        })
    return in_maps


def kernel(M, S, flow):
    nc = _get_nc()
    in_maps = _prep_inputs(M, S, flow)
    res = run_bass_kernel_spmd(nc, in_maps, core_ids=list(range(NCORES)))
    tot = 0.0
    for r in res.results:
        tot += r["gaccA"].astype(np.float64).sum()
        tot += r["gaccB"].astype(np.float64).sum()
    return np.float32(tot / (D * H * W))
